# revision 25
# baseline (speedup 1.0000x reference)
"""BQQ linear inference kernel for 8 Trainium2 NeuronCores.

Math: after activation quantization, the whole BQQ op is linear in the
quantized input, so all four correction terms fold into one weight matrix:

    out[b, (j,m)] = act_scale * (X_int[b, (k,n)] @ W[(k,n), (j,m)]) + bias

where X_int = clip(round(x / act_scale), -127, 127) and W is a pure function
of the weights (Y_sign/Z_sign/scales/A) computed on the host (offline weight
folding).  The device kernel per core:
  1. DMA x^T (fp32, replicated) + its W shard (bf16) in.
  2. global max/min reduce -> act_scale on device.
  3. quantize via the fp32 magic-number round (matches jnp.round's RNE).
  4. 128-contraction GEMM accumulating over k in PSUM.
  5. scale + bias epilogue, DMA out.

Sharding: tensor-parallel over the j (output block) dim, 4 of 32 j-blocks per
core.  Per-core HBM traffic ~13.2 MB (x 8MB fp32 + W 4.2MB bf16 + out 1MB).
"""

import numpy as np
import ml_dtypes

import concourse.bass as bass
import concourse.bacc as bacc
import concourse.mybir as mybir
from concourse.tile import TileContext
from concourse.bass_utils import run_bass_kernel_spmd

F32 = mybir.dt.float32
BF16 = mybir.dt.bfloat16

P_, J, K, M, L, N = 2, 32, 32, 128, 16, 128
B = 512                  # tokens
NCORES = 8
JLOC = J // NCORES       # 4 j-blocks per core
CPJ = JLOC * M           # 512 output cols per core
MAGIC = 12582912.0       # 1.5 * 2**23: fp32 addend that forces RNE to integer
QMAX = 127.0
NCH = 8                  # x DMA chunks (4 k-slices each)
QC = 4                   # k-slices per quantize chunk

_CACHE = {}


def _build_bass():
    nc = bacc.Bacc()
    xt_d = nc.declare_dram_parameter("xt", [N, K * B], F32, isOutput=False)
    w_d = nc.declare_dram_parameter("wgt", [N, K * CPJ], BF16, isOutput=False)
    b_d = nc.declare_dram_parameter("bias", [128, CPJ], F32, isOutput=False)
    out_d = nc.declare_dram_parameter("out", [B, CPJ], F32, isOutput=True)
    sb_d = nc.dram_tensor("sbounce", [2, 128], F32)

    AX = mybir.AxisListType.X
    OP = mybir.AluOpType

    with TileContext(nc) as tc:
        with tc.tile_pool(name="big", bufs=1) as big, \
             tc.tile_pool(name="sm", bufs=1) as sm, \
             tc.tile_pool(name="qtmp", bufs=3) as qtmp, \
             tc.tile_pool(name="ot", bufs=3) as ot, \
             tc.tile_pool(name="psum", bufs=1, space="PSUM") as pp:
            xf = big.tile([N, K * B], F32)        # x^T fp32, 64KB/partition
            xi = big.tile([N, K * B], BF16)       # quantized x^T
            wt = big.tile([N, K * CPJ], BF16)     # folded weights
            bias_t = sm.tile([128, CPJ], F32)
            stat = sm.tile([128, 2 * NCH], F32)   # max partials | -min partials
            mrow = sm.tile([1, 2 * NCH], F32)
            gmx = sm.tile([1, 1], F32)
            gng = sm.tile([1, 1], F32)
            rng1 = sm.tile([1, 1], F32)
            scl1 = sm.tile([1, 1], F32)
            iscl1 = sm.tile([1, 1], F32)
            ones_row = sm.tile([1, 128], F32)
            srow = sm.tile([1, 256], F32)
            scl = sm.tile([128, 1], F32)
            iscl = sm.tile([128, 1], F32)
            scl2 = sm.tile([128, 1], F32)
            iscl2 = sm.tile([128, 1], F32)
            magic_t = sm.tile([128, 1], F32)
            nc.vector.memset(magic_t[:], MAGIC)
            nc.vector.memset(ones_row[:], 1.0)

            # Phase A: stream x^T in, computing per-chunk max/min partials.
            CW = (K // NCH) * B
            for c in range(NCH):
                sl = slice(c * CW, (c + 1) * CW)
                nc.sync.dma_start(out=xf[:, sl], in_=xt_d[:, sl])
                nc.vector.tensor_reduce(stat[:, c:c + 1], xf[:, sl], AX, OP.max)
                nc.vector.tensor_reduce(stat[:, NCH + c:NCH + c + 1], xf[:, sl],
                                        AX, OP.min, negate=True)
            nc.sync.dma_start(out=bias_t[:], in_=b_d[:])
            for k in range(K):
                ks = slice(k * CPJ, (k + 1) * CPJ)
                nc.sync.dma_start(out=wt[:, ks], in_=w_d[:, ks])

            # Phase B: finalize act_scale on device.  Cross-partition step via
            # gpsimd C-axis reduce on the tiny [128, 16] partial tile.
            nc.gpsimd.tensor_reduce(mrow[:], stat[:], mybir.AxisListType.C,
                                    OP.max)
            nc.vector.tensor_reduce(gmx[:], mrow[0:1, 0:NCH], AX, OP.max)
            nc.vector.tensor_reduce(gng[:], mrow[0:1, NCH:2 * NCH], AX, OP.max)
            nc.vector.tensor_add(rng1[:], gmx[:], gng[:])    # gmax - gmin
            nc.vector.tensor_scalar(out=scl1[:], in0=rng1[:],
                                    scalar1=1.0 / (2.0 * QMAX), scalar2=1e-8,
                                    op0=OP.mult, op1=OP.max)
            nc.vector.reciprocal(iscl1[:], scl1[:])
            # replicate the two scalars across a row, then DMA-scatter down
            # the partition dim to get per-partition [128,1] scale vectors.
            nc.vector.tensor_scalar(out=srow[:, 0:128], in0=ones_row[:],
                                    scalar1=scl1[:], scalar2=None,
                                    op0=OP.mult)
            nc.vector.tensor_scalar(out=srow[:, 128:256], in0=ones_row[:],
                                    scalar1=iscl1[:], scalar2=None,
                                    op0=OP.mult)
            # partition-broadcast the two scalars via a DRAM bounce: the row
            # goes out linearly, then comes back as a [128,1] column load.
            nc.gpsimd.dma_start(out=sb_d[0:1, :], in_=srow[0:1, 0:128])
            nc.gpsimd.dma_start(out=sb_d[1:2, :], in_=srow[0:1, 128:256])
            nc.gpsimd.dma_start(out=scl[:],
                                in_=sb_d[0:1, :].rearrange("a b -> b a"))
            nc.gpsimd.dma_start(out=iscl[:],
                                in_=sb_d[1:2, :].rearrange("a b -> b a"))
            # funnel through DVE so every downstream consumer sees at most
            # two distinct wait semaphores (HW embedded-wait limits).
            nc.vector.tensor_copy(scl2[:], scl[:])
            nc.vector.tensor_copy(iscl2[:], iscl[:])

            # Phase C: quantize per chunk; GEMM accumulates over k per b-block.
            psums = [pp.tile([128, CPJ], F32, name=f"psum{i}", tag=f"psum{i}")
                     for i in range(4)]
            for k in range(K):
                if k % QC == 0:
                    qsl = slice(k * B, (k + QC) * B)
                    tq = qtmp.tile([N, QC * B], F32)
                    nc.scalar.activation(tq[:], xf[:, qsl],
                                         mybir.ActivationFunctionType.Identity,
                                         bias=magic_t[:], scale=iscl2[:])
                    nc.vector.tensor_scalar(out=tq[:], in0=tq[:],
                                            scalar1=MAGIC + QMAX,
                                            scalar2=MAGIC - QMAX,
                                            op0=OP.min, op1=OP.max)
                    nc.vector.tensor_scalar(out=xi[:, qsl], in0=tq[:],
                                            scalar1=MAGIC, scalar2=None,
                                            op0=OP.subtract)
                ks = slice(k * CPJ, (k + 1) * CPJ)
                for bb in range(4):
                    nc.tensor.matmul(
                        psums[bb][:],
                        lhsT=xi[:, k * B + bb * 128:k * B + (bb + 1) * 128],
                        rhs=wt[:, ks],
                        start=(k == 0), stop=(k == K - 1))

            # Phase D: scale + bias epilogue, DMA out.
            for bb in range(4):
                o = ot.tile([128, CPJ], F32)
                nc.scalar.activation(o[:], psums[bb][:],
                                     mybir.ActivationFunctionType.Identity,
                                     bias=0.0, scale=scl2[:])
                nc.vector.tensor_add(o[:], o[:], bias_t[:])
                nc.gpsimd.dma_start(out=out_d[bb * 128:(bb + 1) * 128, :],
                                    in_=o[:])
    return nc


def _fold_weights(Y_sign, Z_sign, Y_scale, Z_scale, A):
    """W[j,k,n,m]: everything linear in X folded into one matrix (fp32)."""
    ysc = Y_scale[..., 0, 0].astype(np.float32)      # (p,j,k)
    zsc = Z_scale[..., 0, 0].astype(np.float32)
    a0, a1, a2, a3 = (A[..., i].astype(np.float32) for i in range(4))
    Zs = Z_sign.astype(np.float32)
    Ys = Y_sign.astype(np.float32)
    # out1: sum_{p,l} a0*ysc*zsc * Z[l,n] * Y[m,l]  -> (j,k,n,m)
    t1 = np.einsum('pjkln,pjkml->pjknm', Zs, Ys, optimize=True)
    W = np.einsum('pjk,pjknm->jknm', a0 * ysc * zsc, t1, optimize=True)
    # out2: B_coef[j,k,m] broadcast over n
    Ysum = Ys.sum(-1) * ysc[..., None]               # (p,j,k,m)
    W += np.einsum('pjk,pjkm->jkm', a1, Ysum)[:, :, None, :]
    # out3: sum_p a2*zsc*Zsum[n] broadcast over m
    Zsum = Zs.sum(-2) * zsc[..., None]               # (p,j,k,n)
    W += np.einsum('pjk,pjkn->jkn', a2, Zsum)[:, :, :, None]
    # out4: D_coef[j,k] broadcast over n,m
    W += a3.sum(0)[:, :, None, None]
    return W


def _prepare(inputs):
    x = np.asarray(inputs["input"], dtype=np.float32)
    W = _fold_weights(np.asarray(inputs["Y_sign"], np.float32),
                      np.asarray(inputs["Z_sign"], np.float32),
                      np.asarray(inputs["Y_scale"], np.float32),
                      np.asarray(inputs["Z_scale"], np.float32),
                      np.asarray(inputs["A"], np.float32))
    bias = np.asarray(inputs["bias"], np.float32)

    # x^T layout [n, (k, b)]
    xt = np.ascontiguousarray(
        x.reshape(B, K, N).transpose(2, 1, 0).reshape(N, K * B))

    in_maps = []
    for cid in range(NCORES):
        Wc = W[cid * JLOC:(cid + 1) * JLOC]          # [jl,k,n,m]
        wgt = np.ascontiguousarray(
            Wc.transpose(2, 1, 0, 3).reshape(N, K * CPJ)).astype(
                ml_dtypes.bfloat16)                  # [n, (k, jl, m)]
        bc = np.ascontiguousarray(np.broadcast_to(
            bias[cid * CPJ:(cid + 1) * CPJ].reshape(1, CPJ), (128, CPJ)))
        in_maps.append({"xt": xt, "wgt": wgt, "bias": bc})
    return in_maps


def _run(inputs, trace=False):
    if "nc" not in _CACHE:
        nc = _build_bass()
        nc.finalize()          # run bacc passes (reg alloc, wait splitting)
        _CACHE["nc"] = nc
    nc = _CACHE["nc"]
    in_maps = _prepare(inputs)
    res = run_bass_kernel_spmd(nc, in_maps, list(range(NCORES)), trace=trace)
    out = np.concatenate([res.results[c]["out"] for c in range(NCORES)], axis=1)
    out = out.reshape(1, B, J * M).astype(np.float32)
    return out, res


def kernel(**inputs) -> np.ndarray:
    out, _ = _run(inputs, trace=False)
    return out
